# revision 14
# baseline (speedup 1.0000x reference)
"""Cross-attention kernel for Trainium2 (8 NeuronCores, SPMD).

Reference computation (B=4, Sq=1024, Sk=2048, D=1024, H=16, dh=64):
    q  = x @ Wq + bq                         [B,Sq,D]  -> heads
    kv = ctx @ Wkv + bkv                     [B,Sk,2D] -> k, v heads
    s  = q k^T / sqrt(dh) + mask ; p = softmax(s)
    a  = p v  (merge heads)                  [B,Sq,D]
    out= a @ Wp + bp

Sharding: core c handles batch b=c//2 and query half h=c%2 (rows
[512h, 512h+512) of x[b]) with ALL 16 heads, so each core emits a
complete, disjoint [512, 1024] slice of the output — no host-side
reduction, just a reshape.  Each core uploads only its own ctx HALF
(rows [1024h, 1024h+1024) of ctx[b]); the k/v halves are exchanged
between the two cores of a batch with an on-device pairwise HBM
AllGather (replica groups {0,1},{2,3},{4,5},{6,7}).

The wall-clock bottleneck in this environment is the axon tunnel
(~40-75 MB/s host<->device, serialized, not duplex), so the design
minimizes per-call transfer (24 MB up + 8 MB down):
  - x and ctx ship in natural row-major layout as f16 (8 MB + 16 MB);
    transposes happen on-device on the TensorE (identity matmul).
  - weights/biases/identity ship once and stay device-resident as jax
    arrays across calls (keyed by a full checksum of the weights).
  - the jit(shard_map(bass_exec)) callable is built ONCE and cached
    (the stock run_bass_kernel_spmd builds a fresh jit per call, which
    re-pays XLA compile/NEFF load and re-uploads everything).
  - PJRT donation buffers are the previous call's device outputs (the
    kernel writes every output element, so contents are irrelevant);
    a jitted on-device zeros-maker seeds the first call.  Nothing is
    uploaded for donation.
  - output is a [4096, 1024] f16 shard-concat, fetched once (8 MB).

Device program (per core), all f16 matmul operands / f32 PSUM:
  xT   = transpose(x_half)   via TensorE identity-matmul  [1024, 512]
  cxT  = transpose(ctx_half)                               [1024, 1024]
  qT   = Wq^T @ xT + bq       8 tiles [128, 512]   (tile t = 2 heads)
  kT_h = Wk^T @ cxT + bk      8 tiles [128, 1024]  (own sk half)
  v_h  = cx @ Wv + bv         8 tiles [128 skh, 1024]
  kv AllGather within the core pair -> full kT [128, 2048] x8,
    v [128, 1024] x16
  per head-pair hp: sT = kT-chunk^T qT -> exp(s/8+mask) -> p (f16)
    a^T += v-cols^T p  (PSUM quadrant accumulation over 16 sk chunks)
    colsums via ones^T @ running-f16-sum, reciprocal, broadcast mul
  out  = a @ Wp + bp          natural [512, 1024] via lhsT = a^T tiles
"""

import sys

import numpy as np

if "/opt/trn_rl_repo" not in sys.path:
    sys.path.insert(0, "/opt/trn_rl_repo")

N_HEAD = 16
B, SQ, SK, D = 4, 1024, 2048, 1024
DH = D // N_HEAD          # 64
NCORES = 8
SQH = SQ // 2             # 512 query rows per core
NKC = SK // 128           # 16 sk chunks

_CACHE = {}


def _build_program():
    """Trace the Bass/Tile program once; returns nc."""
    import concourse.bass as bass  # noqa: F401
    import concourse.mybir as mybir
    import concourse.tile as tile
    from concourse import bacc
    from contextlib import ExitStack

    f16 = mybir.dt.float16
    f32 = mybir.dt.float32
    AF = mybir.ActivationFunctionType

    nc = bacc.Bacc(
        "TRN2",
        target_bir_lowering=False,
        debug=False,
        enable_asserts=False,
        num_devices=NCORES,
    )

    SKH = SK // 2
    x_d = nc.dram_tensor("x", [SQH, D], f16, kind="ExternalInput")
    cx_d = nc.dram_tensor("cx", [SKH, D], f16, kind="ExternalInput")
    wq_d = nc.dram_tensor("wq", [D, D], f16, kind="ExternalInput")
    wk_d = nc.dram_tensor("wk", [D, D], f16, kind="ExternalInput")
    wv_d = nc.dram_tensor("wv", [D, D], f16, kind="ExternalInput")
    wp_d = nc.dram_tensor("wp", [D, D], f16, kind="ExternalInput")
    bq_d = nc.dram_tensor("bq_t", [128, 8], f32, kind="ExternalInput")
    bk_d = nc.dram_tensor("bk_t", [128, 8], f32, kind="ExternalInput")
    bv_d = nc.dram_tensor("bv_bc", [128, D], f32, kind="ExternalInput")
    bp_d = nc.dram_tensor("bp_bc", [128, D], f32, kind="ExternalInput")
    mk_d = nc.dram_tensor("mask_t", [128, NKC], f32, kind="ExternalInput")
    id_d = nc.dram_tensor("ident", [128, 128], f16, kind="ExternalInput")
    om_d = nc.dram_tensor("ones_m", [128, 128], f16, kind="ExternalInput")
    out_d = nc.dram_tensor("out", [SQH, D], f16, kind="ExternalOutput")

    with tile.TileContext(nc) as tc, ExitStack() as stk:
        persist = stk.enter_context(tc.tile_pool(name="persist", bufs=1))
        # persistent activations for attention + c_proj
        q_sb = [persist.tile([128, SQH], f16, tag=f"q{t}", name=f"q{t}")
                for t in range(8)]
        k_sb = [persist.tile([128, SK], f16, tag=f"k{t}", name=f"k{t}")
                for t in range(8)]
        v_sb = [persist.tile([128, D], f16, tag=f"v{s}", name=f"v{s}")
                for s in range(NKC)]
        a_sb = [persist.tile([128, SQH], f16, tag=f"a{t}", name=f"a{t}")
                for t in range(8)]
        wp_sb = [persist.tile([128, 512], f16, tag=f"wp{t}{dc}",
                              name=f"wp{t}{dc}")
                 for t in range(8) for dc in range(2)]
        bp_sb = persist.tile([128, D], f32, tag="bp", name="bp_sb")
        mk_sb = persist.tile([128, NKC], f32, tag="mk", name="mk_sb")
        om_sb = persist.tile([128, 128], f16, tag="onesm", name="om_sb")
        for t in range(8):
            for dc in range(2):
                nc.sync.dma_start(
                    wp_sb[2 * t + dc][:, :],
                    wp_d[128 * t:128 * (t + 1), 512 * dc:512 * (dc + 1)])
        nc.sync.dma_start(bp_sb[:, :], bp_d[:, :])
        nc.sync.dma_start(mk_sb[:, :], mk_d[:, :])
        nc.sync.dma_start(om_sb[:, :], om_d[:, :])

        # ---------------- Phase A: transposes + projections ----------------
        stageA = ExitStack()
        mpool = stageA.enter_context(tc.tile_pool(name="midA", bufs=1))
        psT = stageA.enter_context(
            tc.tile_pool(name="psT", bufs=2, space="PSUM"))
        psP = stageA.enter_context(
            tc.tile_pool(name="psP", bufs=2, space="PSUM"))

        id_sb = mpool.tile([128, 128], f16, tag="ident", name="id_sb")
        bq_sb = mpool.tile([128, 8], f32, tag="bq", name="bq_sb")
        bk_sb = mpool.tile([128, 8], f32, tag="bk", name="bk_sb")
        bv_sb = mpool.tile([128, D], f32, tag="bv", name="bv_sb")
        nc.sync.dma_start(id_sb[:, :], id_d[:, :])
        nc.sync.dma_start(bq_sb[:, :], bq_d[:, :])
        nc.sync.dma_start(bk_sb[:, :], bk_d[:, :])
        nc.sync.dma_start(bv_sb[:, :], bv_d[:, :])

        xT_sb = [mpool.tile([128, SQH], f16, tag=f"xT{j}", name=f"xT{j}")
                 for j in range(8)]
        cxT_sb = [mpool.tile([128, SKH], f16, tag=f"cxT{j}", name=f"cxT{j}")
                  for j in range(8)]

        # DRAM bounce buffers for the pairwise kv AllGather
        dpool = stageA.enter_context(
            tc.tile_pool(name="dramA", bufs=1, space="DRAM"))
        kv_in = dpool.tile([2 * SKH, D], f16, tag="kvin", name="kv_in")
        kv_out = dpool.tile([4 * SKH, D], f16, tag="kvout", name="kv_out")

        # sub-stage A1: raw x/cx tiles, freed after the transposes
        stageA1 = ExitStack()
        rpool = stageA1.enter_context(tc.tile_pool(name="rawA", bufs=1))
        x_sb = [rpool.tile([128, D], f16, tag=f"x{i}", name=f"x{i}")
                for i in range(4)]
        cx_sb = [rpool.tile([128, D], f16, tag=f"cx{i}", name=f"cx{i}")
                 for i in range(8)]
        for i in range(4):
            nc.sync.dma_start(x_sb[i][:, :], x_d[128 * i:128 * (i + 1), :])
        for i in range(8):
            nc.sync.dma_start(cx_sb[i][:, :], cx_d[128 * i:128 * (i + 1), :])
        for j in range(8):
            for i in range(4):
                pt = psT.tile([128, 128], f16, tag="tp", name="tp")
                nc.tensor.transpose(
                    pt[:, :], x_sb[i][:, 128 * j:128 * (j + 1)], id_sb[:, :])
                nc.vector.tensor_copy(
                    xT_sb[j][:, 128 * i:128 * (i + 1)], pt[:, :])
        for j in range(8):
            for i in range(8):
                pt = psT.tile([128, 128], f16, tag="tp", name="tp")
                nc.tensor.transpose(
                    pt[:, :], cx_sb[i][:, 128 * j:128 * (j + 1)], id_sb[:, :])
                nc.vector.tensor_copy(
                    cxT_sb[j][:, 128 * i:128 * (i + 1)], pt[:, :])
        stageA1.close()

        # sub-stage A2: projection weights
        stageA2 = ExitStack()
        wpool = stageA2.enter_context(tc.tile_pool(name="wA", bufs=1))
        wq_sb = [wpool.tile([128, D], f16, tag=f"wq{d}", name=f"wq{d}")
                 for d in range(8)]
        wk_sb = [wpool.tile([128, D], f16, tag=f"wk{d}", name=f"wk{d}")
                 for d in range(8)]
        wv_sb = [wpool.tile([128, D], f16, tag=f"wv{d}", name=f"wv{d}")
                 for d in range(8)]
        for d in range(8):
            nc.sync.dma_start(wq_sb[d][:, :], wq_d[128 * d:128 * (d + 1), :])
            nc.sync.dma_start(wk_sb[d][:, :], wk_d[128 * d:128 * (d + 1), :])
            nc.sync.dma_start(wv_sb[d][:, :], wv_d[128 * d:128 * (d + 1), :])

        # qT tiles [128, 512]: tile t = features 128t..128t+127
        for t in range(8):
            ps = psP.tile([128, 512], f32, tag="proj", name="proj_ps")
            for d in range(8):
                nc.tensor.matmul(
                    ps[:, :],
                    lhsT=wq_sb[d][:, 128 * t:128 * (t + 1)],
                    rhs=xT_sb[d][:, :],
                    start=(d == 0), stop=(d == 7),
                )
            nc.vector.tensor_scalar_add(
                q_sb[t][:, :], ps[:, :], bq_sb[:, t:t + 1])
        # own-half kT tiles [128, 1024] -> DRAM bounce rows [128t, :]
        hpool = stageA2.enter_context(tc.tile_pool(name="halves", bufs=2))
        for t in range(8):
            kh = hpool.tile([128, SKH], f16, tag="kh", name="kh")
            for n in range(2):
                ps = psP.tile([128, 512], f32, tag="proj", name="proj_ps")
                for d in range(8):
                    nc.tensor.matmul(
                        ps[:, :],
                        lhsT=wk_sb[d][:, 128 * t:128 * (t + 1)],
                        rhs=cxT_sb[d][:, 512 * n:512 * (n + 1)],
                        start=(d == 0), stop=(d == 7),
                    )
                nc.vector.tensor_scalar_add(
                    kh[:, 512 * n:512 * (n + 1)], ps[:, :],
                    bk_sb[:, t:t + 1])
            nc.sync.dma_start(kv_in[128 * t:128 * (t + 1), :], kh[:, :])
        # own-half v tiles [128 skh, 1024] -> bounce rows [SKH + 128s, :]
        for s in range(8):
            vh = hpool.tile([128, D], f16, tag="vh", name="vh")
            for dc in range(2):
                ps = psP.tile([128, 512], f32, tag="proj", name="proj_ps")
                for d in range(8):
                    nc.tensor.matmul(
                        ps[:, :],
                        lhsT=cxT_sb[d][:, 128 * s:128 * (s + 1)],
                        rhs=wv_sb[d][:, 512 * dc:512 * (dc + 1)],
                        start=(d == 0), stop=(d == 7),
                    )
                nc.vector.tensor_add(
                    vh[:, 512 * dc:512 * (dc + 1)], ps[:, :],
                    bv_sb[:, 512 * dc:512 * (dc + 1)])
            nc.sync.dma_start(kv_in[SKH + 128 * s:SKH + 128 * (s + 1), :],
                              vh[:, :])

        # pairwise exchange: cores (2b, 2b+1) gather both kv halves
        nc.gpsimd.collective_compute(
            "AllGather",
            mybir.AluOpType.bypass,
            replica_groups=[[0, 1], [2, 3], [4, 5], [6, 7]],
            ins=[kv_in.opt()],
            outs=[kv_out.opt()],
        )

        # load back full kT [128, 2048] and v [128 sk, 1024] tiles
        for t in range(8):
            for r in range(2):
                nc.sync.dma_start(
                    k_sb[t][:, SKH * r:SKH * (r + 1)],
                    kv_out[2 * SKH * r + 128 * t:
                           2 * SKH * r + 128 * (t + 1), :])
        for s in range(NKC):
            r, sl = s // 8, s % 8
            nc.sync.dma_start(
                v_sb[s][:, :],
                kv_out[2 * SKH * r + SKH + 128 * sl:
                       2 * SKH * r + SKH + 128 * (sl + 1), :])

        stageA2.close()
        stageA.close()   # free xT/cxT/weights SBUF + psum pools

        # ---------------- Phase B: attention per head-pair ----------------
        stageB = ExitStack()
        work = stageB.enter_context(tc.tile_pool(name="work", bufs=3))
        norm = stageB.enter_context(tc.tile_pool(name="norm", bufs=1))
        psS = stageB.enter_context(
            tc.tile_pool(name="psS", bufs=3, space="PSUM"))
        psAcc = stageB.enter_context(
            tc.tile_pool(name="psAcc", bufs=2, space="PSUM"))
        psSum = stageB.enter_context(
            tc.tile_pool(name="psSum", bufs=2, space="PSUM"))

        for hp in range(8):
            acc = psAcc.tile([128, SQH], f32, tag="apsum", name="apsum")
            sums = work.tile([128, 2 * SQH], f16, tag="sums", name="sums",
                             bufs=2)
            for c in range(NKC):
                pt = work.tile([128, 2 * SQH], f16, tag="p", name="ptile")
                for a in range(2):
                    sc = psS.tile([128, SQH], f32, tag="scores",
                                  name="scores")
                    nc.tensor.matmul(
                        sc[:, :],
                        lhsT=k_sb[hp][64 * a:64 * (a + 1),
                                      128 * c:128 * (c + 1)],
                        rhs=q_sb[hp][64 * a:64 * (a + 1), :],
                        tile_position=(64 * a, 0),
                        start=True, stop=True,
                    )
                    nc.scalar.activation(
                        pt[:, SQH * a:SQH * (a + 1)], sc[:, :],
                        AF.Exp, bias=mk_sb[:, c:c + 1], scale=0.125,
                    )
                if c == 0:
                    nc.vector.tensor_copy(sums[:, :], pt[:, :])
                else:
                    nc.vector.tensor_add(sums[:, :], sums[:, :], pt[:, :])
                for a in range(2):
                    nc.tensor.matmul(
                        acc[64 * a:64 * (a + 1), :],
                        lhsT=v_sb[c][:, 64 * (2 * hp + a):
                                     64 * (2 * hp + a + 1)],
                        rhs=pt[:, SQH * a:SQH * (a + 1)],
                        tile_position=(0, 64 * a),
                        start=(c == 0), stop=(c == NKC - 1),
                    )
            # normalization: R = 1 / colsum(exp) broadcast to partitions
            r_bc = norm.tile([128, 2 * SQH], f32, tag="rbc", name="rbc")
            for j in range(2):
                sp = psSum.tile([128, 512], f32, tag="sum_ps", name="sum_ps")
                nc.tensor.matmul(
                    sp[:, :], lhsT=om_sb[:, :],
                    rhs=sums[:, 512 * j:512 * (j + 1)],
                    start=True, stop=True,
                )
                nc.vector.reciprocal_approx_fast(
                    r_bc[:, 512 * j:512 * (j + 1)], sp[:, :])
            for a in range(2):
                nc.vector.tensor_mul(
                    a_sb[hp][64 * a:64 * (a + 1), :],
                    acc[64 * a:64 * (a + 1), :],
                    r_bc[64 * a:64 * (a + 1), SQH * a:SQH * (a + 1)],
                )
        stageB.close()

        # ---------------- Phase C: c_proj, natural [sq, d] layout ---------
        stageC = ExitStack()
        cpool = stageC.enter_context(tc.tile_pool(name="stageC", bufs=2))
        psO = stageC.enter_context(
            tc.tile_pool(name="psO", bufs=2, space="PSUM"))
        for sqc in range(4):
            ot = cpool.tile([128, D], f16, tag="outT", name="ot")
            for dc in range(2):
                ps = psO.tile([128, 512], f32, tag="ops", name="ops")
                for hp in range(8):
                    nc.tensor.matmul(
                        ps[:, :],
                        lhsT=a_sb[hp][:, 128 * sqc:128 * (sqc + 1)],
                        rhs=wp_sb[2 * hp + dc][:, :],
                        start=(hp == 0), stop=(hp == 7),
                    )
                nc.vector.tensor_add(
                    ot[:, 512 * dc:512 * (dc + 1)], ps[:, :],
                    bp_sb[:, 512 * dc:512 * (dc + 1)])
            nc.sync.dma_start(out_d[128 * sqc:128 * (sqc + 1), :], ot[:, :])
        stageC.close()

    nc.compile()
    return nc


class _Runner:
    """Cached jit(shard_map(bass_exec)) with device-resident weights."""

    def __init__(self, nc):
        import jax
        import concourse.mybir as mybir
        from concourse import bass2jax
        from jax.sharding import Mesh, NamedSharding, PartitionSpec
        from jax.experimental.shard_map import shard_map

        bass2jax.install_neuronx_cc_hook()
        self.jax = jax
        self.nc = nc

        partition_name = (nc.partition_id_tensor.name
                          if nc.partition_id_tensor else None)
        in_names, out_names, out_avals = [], [], []
        for alloc in nc.m.functions[0].allocations:
            if not isinstance(alloc, mybir.MemoryLocationSet):
                continue
            name = alloc.memorylocations[0].name
            if alloc.kind == "ExternalInput":
                if name != partition_name:
                    in_names.append(name)
            elif alloc.kind == "ExternalOutput":
                out_names.append(name)
                out_avals.append(jax.core.ShapedArray(
                    tuple(alloc.tensor_shape), mybir.dt.np(alloc.dtype)))
        self.in_names = in_names
        self.out_names = out_names
        self.out_avals = out_avals
        n_params = len(in_names)
        bind_names = tuple(in_names + out_names +
                           ([partition_name] if partition_name else []))

        devices = jax.devices()[:NCORES]
        self.mesh = Mesh(np.asarray(devices), ("core",))
        self.shard = NamedSharding(self.mesh, PartitionSpec("core"))

        def _body(*args):
            operands = list(args)
            if partition_name is not None:
                operands.append(bass2jax.partition_id_tensor())
            outs = bass2jax._bass_exec_p.bind(
                *operands,
                out_avals=tuple(out_avals),
                in_names=bind_names,
                out_names=tuple(out_names),
                lowering_input_output_aliases=(),
                sim_require_finite=True,
                sim_require_nnan=True,
                nc=nc,
            )
            return tuple(outs)

        donate = tuple(range(n_params, n_params + len(out_names)))
        self.sharded = jax.jit(
            shard_map(_body, mesh=self.mesh,
                      in_specs=(PartitionSpec("core"),) * (n_params +
                                                           len(out_names)),
                      out_specs=(PartitionSpec("core"),) * len(out_names),
                      check_rep=False),
            donate_argnums=donate, keep_unused=True)

        import jax.numpy as jnp
        zero_shards = [NamedSharding(self.mesh, PartitionSpec("core"))
                       for _ in out_avals]

        def _mk_zeros():
            return tuple(
                jnp.zeros((NCORES * a.shape[0],) + tuple(a.shape[1:]),
                          a.dtype)
                for a in out_avals)

        self.mk_zeros = jax.jit(_mk_zeros, out_shardings=tuple(zero_shards))
        self._prev_outs = None

    def put(self, arr):
        """Host concat array -> device-resident sharded jax array."""
        return self.jax.device_put(arr, self.shard)

    def run(self, operands):
        """operands: dict name -> array (jax device or numpy concat)."""
        # Donation buffers: reuse the previous call's (already-fetched)
        # device outputs when available — the kernel overwrites every
        # element, so contents are irrelevant; else make zeros on-device.
        prev = self._prev_outs
        self._prev_outs = None
        if prev is None or any(p.is_deleted() for p in prev):
            prev = list(self.mk_zeros())
        args = [operands[n] for n in self.in_names] + prev
        outs = self.sharded(*args)
        self._prev_outs = list(outs)
        return outs


def _checksum(arr):
    """Full-integrity checksum (uint32-view sum) — cheap vs the transfer."""
    a = np.ascontiguousarray(arr)
    return (a.shape, a.dtype.str,
            int(a.view(np.uint32).sum(dtype=np.uint64)))


def _prep_weights(runner, Wq, bq, Wkv, bkv, Wp, bp):
    """Device-resident per-core weight shards (identical on all cores)."""
    f16, f32 = np.float16, np.float32

    def rep(a):   # replicate per core along axis0 for shard_map concat
        return np.ascontiguousarray(
            np.broadcast_to(a, (NCORES,) + a.shape)).reshape(
                (NCORES * a.shape[0],) + a.shape[1:])

    wq = Wq.astype(f16)
    wk = Wkv[:, :D].astype(f16)
    wv = Wkv[:, D:].astype(f16)
    wp = Wp.astype(f16)
    bq_t = np.ascontiguousarray(bq.astype(f32).reshape(8, 128).T)
    bk_t = np.ascontiguousarray(bkv[:D].astype(f32).reshape(8, 128).T)
    bv_bc = np.ascontiguousarray(
        np.broadcast_to(bkv[D:].astype(f32), (128, D)))
    bp_bc = np.ascontiguousarray(
        np.broadcast_to(bp.astype(f32), (128, D)))
    ident = np.eye(128, dtype=f16)
    ones_m = np.ones((128, 128), dtype=f16)
    host = {
        "wq": rep(wq), "wk": rep(wk), "wv": rep(wv), "wp": rep(wp),
        "bq_t": rep(bq_t), "bk_t": rep(bk_t), "bv_bc": rep(bv_bc),
        "bp_bc": rep(bp_bc), "ident": rep(ident), "ones_m": rep(ones_m),
    }
    return {k: runner.put(v) for k, v in host.items()}


def _pool():
    from concurrent.futures import ThreadPoolExecutor
    if "pool" not in _CACHE:
        _CACHE["pool"] = ThreadPoolExecutor(8)
    return _CACHE["pool"]


def _prep_acts(runner, x, ctx, attention_mask):
    """Per-call activations: natural layout f16, zero-copy reshapes.
    The big f32->f16 casts run chunked on a persistent thread pool."""
    f16, f32 = np.float16, np.float32
    x = np.asarray(x)
    ctx = np.asarray(ctx)
    if x.dtype == f16 and ctx.dtype == f16:
        x16, cx16 = x, ctx
    else:
        x16 = np.empty(x.shape, f16)
        cx16 = np.empty(ctx.shape, f16)

        def cast(i):
            if i < 2:                      # x in 2 chunks
                s = slice(i * 2, (i + 1) * 2)
                x16[s] = x[s]
            else:                          # ctx in 4 chunks
                s = slice(i - 2, i - 1)
                cx16[s] = ctx[s]

        list(_pool().map(cast, range(6)))
    m = np.asarray(attention_mask, dtype=f32).reshape(B, SK)
    mask_t = np.ascontiguousarray(
        m.reshape(B, NKC, 128).transpose(0, 2, 1))       # [4,128,16]
    concat_mask = np.ascontiguousarray(
        mask_t[[0, 0, 1, 1, 2, 2, 3, 3]]).reshape(NCORES * 128, NKC)
    return {"x": x16.reshape(NCORES * SQH, D),
            "cx": cx16.reshape(NCORES * (SK // 2), D),
            "mask_t": concat_mask}


def kernel(x, ctx, attention_mask, Wq, bq, Wkv, bkv, Wp, bp, **_ignored):
    x = np.asarray(x); ctx = np.asarray(ctx)
    attention_mask = np.asarray(attention_mask)
    Wq = np.asarray(Wq); bq = np.asarray(bq)
    Wkv = np.asarray(Wkv); bkv = np.asarray(bkv)
    Wp = np.asarray(Wp); bp = np.asarray(bp)

    if "nc" not in _CACHE:
        _CACHE["nc"] = _build_program()
    nc = _CACHE["nc"]
    if "runner" not in _CACHE:
        _CACHE["runner"] = _Runner(nc)
    runner = _CACHE["runner"]

    wkey = tuple(_checksum(a) for a in (Wq, bq, Wkv, bkv, Wp, bp))
    if _CACHE.get("wkey") != wkey:
        _CACHE["weights"] = _prep_weights(runner, Wq, bq, Wkv, bkv, Wp, bp)
        _CACHE["wkey"] = wkey

    operands = dict(_CACHE["weights"])
    operands.update(_prep_acts(runner, x, ctx, attention_mask))
    outs = runner.run(operands)
    try:
        # start the D2H stream the moment the device finishes, instead of
        # paying a client round trip after readiness
        outs[0].copy_to_host_async()
    except Exception:
        pass
    # fetch per-shard on threads; the f16->f32 cast of each shard overlaps
    # the remaining shard transfers
    out = np.empty((B, SQ, D), np.float32)
    flat = out.reshape(NCORES, SQH, D)
    try:
        shards = sorted(outs[0].addressable_shards,
                        key=lambda s: s.index[0].start or 0)
        if len(shards) != NCORES:
            raise ValueError("unexpected shard count")

        def fetch_one(i):
            flat[i] = np.asarray(shards[i].data)     # f16 -> f32 in place

        list(_pool().map(fetch_one, range(NCORES)))
    except Exception:
        out16 = np.asarray(outs[0])                      # fallback
        flat[:] = out16.reshape(NCORES, SQH, D)
    return out


# revision 15
# speedup vs baseline: 1.0274x; 1.0274x over previous
"""Cross-attention kernel for Trainium2 (8 NeuronCores, SPMD).

Reference computation (B=4, Sq=1024, Sk=2048, D=1024, H=16, dh=64):
    q  = x @ Wq + bq                         [B,Sq,D]  -> heads
    kv = ctx @ Wkv + bkv                     [B,Sk,2D] -> k, v heads
    s  = q k^T / sqrt(dh) + mask ; p = softmax(s)
    a  = p v  (merge heads)                  [B,Sq,D]
    out= a @ Wp + bp

Sharding: core c handles batch b=c//2 and query half h=c%2 (rows
[512h, 512h+512) of x[b]) with ALL 16 heads, so each core emits a
complete, disjoint [512, 1024] slice of the output — no host-side
reduction, just a reshape.  Each core uploads only its own ctx HALF
(rows [1024h, 1024h+1024) of ctx[b]); the k/v halves are exchanged
between the two cores of a batch with an on-device pairwise HBM
AllGather (replica groups {0,1},{2,3},{4,5},{6,7}).

The wall-clock bottleneck in this environment is the axon tunnel
(~40-75 MB/s host<->device, serialized, not duplex), so the design
minimizes per-call transfer (24 MB up + 8 MB down):
  - x and ctx ship in natural row-major layout as f16 (8 MB + 16 MB);
    transposes happen on-device on the TensorE (identity matmul).
  - weights/biases/identity ship once and stay device-resident as jax
    arrays across calls (keyed by a full checksum of the weights).
  - the jit(shard_map(bass_exec)) callable is built ONCE and cached
    (the stock run_bass_kernel_spmd builds a fresh jit per call, which
    re-pays XLA compile/NEFF load and re-uploads everything).
  - PJRT donation buffers are the previous call's device outputs (the
    kernel writes every output element, so contents are irrelevant);
    a jitted on-device zeros-maker seeds the first call.  Nothing is
    uploaded for donation.
  - output is a [4096, 1024] f16 shard-concat, fetched once (8 MB).

Device program (per core), all f16 matmul operands / f32 PSUM:
  xT   = transpose(x_half)   via TensorE identity-matmul  [1024, 512]
  cxT  = transpose(ctx_half)                               [1024, 1024]
  qT   = Wq^T @ xT + bq       8 tiles [128, 512]   (tile t = 2 heads)
  kT_h = Wk^T @ cxT + bk      8 tiles [128, 1024]  (own sk half)
  v_h  = cx @ Wv + bv         8 tiles [128 skh, 1024]
  kv AllGather within the core pair -> full kT [128, 2048] x8,
    v [128, 1024] x16
  per head-pair hp: sT = kT-chunk^T qT -> exp(s/8+mask) -> p (f16)
    a^T += v-cols^T p  (PSUM quadrant accumulation over 16 sk chunks)
    colsums via ones^T @ running-f16-sum, reciprocal, broadcast mul
  out  = a @ Wp + bp          natural [512, 1024] via lhsT = a^T tiles
"""

import sys

import numpy as np

if "/opt/trn_rl_repo" not in sys.path:
    sys.path.insert(0, "/opt/trn_rl_repo")

N_HEAD = 16
B, SQ, SK, D = 4, 1024, 2048, 1024
DH = D // N_HEAD          # 64
NCORES = 8
SQH = SQ // 2             # 512 query rows per core
NKC = SK // 128           # 16 sk chunks

_CACHE = {}


def _build_program():
    """Trace the Bass/Tile program once; returns nc."""
    import concourse.bass as bass  # noqa: F401
    import concourse.mybir as mybir
    import concourse.tile as tile
    from concourse import bacc
    from contextlib import ExitStack

    f16 = mybir.dt.float16
    f32 = mybir.dt.float32
    AF = mybir.ActivationFunctionType

    nc = bacc.Bacc(
        "TRN2",
        target_bir_lowering=False,
        debug=False,
        enable_asserts=False,
        num_devices=NCORES,
    )

    SKH = SK // 2
    # packed per-core activations: rows [0,512) = x half, [512,1536) = ctx half
    xin_d = nc.dram_tensor("xin", [SQH + SKH, D], f16, kind="ExternalInput")
    wq_d = nc.dram_tensor("wq", [D, D], f16, kind="ExternalInput")
    wk_d = nc.dram_tensor("wk", [D, D], f16, kind="ExternalInput")
    wv_d = nc.dram_tensor("wv", [D, D], f16, kind="ExternalInput")
    wp_d = nc.dram_tensor("wp", [D, D], f16, kind="ExternalInput")
    bq_d = nc.dram_tensor("bq_t", [128, 8], f32, kind="ExternalInput")
    bk_d = nc.dram_tensor("bk_t", [128, 8], f32, kind="ExternalInput")
    bv_d = nc.dram_tensor("bv_bc", [128, D], f32, kind="ExternalInput")
    bp_d = nc.dram_tensor("bp_bc", [128, D], f32, kind="ExternalInput")
    mk_d = nc.dram_tensor("mask_t", [128, NKC], f32, kind="ExternalInput")
    id_d = nc.dram_tensor("ident", [128, 128], f16, kind="ExternalInput")
    om_d = nc.dram_tensor("ones_m", [128, 128], f16, kind="ExternalInput")
    out_d = nc.dram_tensor("out", [SQH, D], f16, kind="ExternalOutput")

    with tile.TileContext(nc) as tc, ExitStack() as stk:
        persist = stk.enter_context(tc.tile_pool(name="persist", bufs=1))
        # persistent activations for attention + c_proj
        q_sb = [persist.tile([128, SQH], f16, tag=f"q{t}", name=f"q{t}")
                for t in range(8)]
        k_sb = [persist.tile([128, SK], f16, tag=f"k{t}", name=f"k{t}")
                for t in range(8)]
        v_sb = [persist.tile([128, D], f16, tag=f"v{s}", name=f"v{s}")
                for s in range(NKC)]
        a_sb = [persist.tile([128, SQH], f16, tag=f"a{t}", name=f"a{t}")
                for t in range(8)]
        wp_sb = [persist.tile([128, 512], f16, tag=f"wp{t}{dc}",
                              name=f"wp{t}{dc}")
                 for t in range(8) for dc in range(2)]
        bp_sb = persist.tile([128, D], f32, tag="bp", name="bp_sb")
        mk_sb = persist.tile([128, NKC], f32, tag="mk", name="mk_sb")
        om_sb = persist.tile([128, 128], f16, tag="onesm", name="om_sb")
        for t in range(8):
            for dc in range(2):
                nc.sync.dma_start(
                    wp_sb[2 * t + dc][:, :],
                    wp_d[128 * t:128 * (t + 1), 512 * dc:512 * (dc + 1)])
        nc.sync.dma_start(bp_sb[:, :], bp_d[:, :])
        nc.sync.dma_start(mk_sb[:, :], mk_d[:, :])
        nc.sync.dma_start(om_sb[:, :], om_d[:, :])

        # ---------------- Phase A: transposes + projections ----------------
        stageA = ExitStack()
        mpool = stageA.enter_context(tc.tile_pool(name="midA", bufs=1))
        psT = stageA.enter_context(
            tc.tile_pool(name="psT", bufs=2, space="PSUM"))
        psP = stageA.enter_context(
            tc.tile_pool(name="psP", bufs=2, space="PSUM"))

        id_sb = mpool.tile([128, 128], f16, tag="ident", name="id_sb")
        bq_sb = mpool.tile([128, 8], f32, tag="bq", name="bq_sb")
        bk_sb = mpool.tile([128, 8], f32, tag="bk", name="bk_sb")
        bv_sb = mpool.tile([128, D], f32, tag="bv", name="bv_sb")
        nc.sync.dma_start(id_sb[:, :], id_d[:, :])
        nc.sync.dma_start(bq_sb[:, :], bq_d[:, :])
        nc.sync.dma_start(bk_sb[:, :], bk_d[:, :])
        nc.sync.dma_start(bv_sb[:, :], bv_d[:, :])

        xT_sb = [mpool.tile([128, SQH], f16, tag=f"xT{j}", name=f"xT{j}")
                 for j in range(8)]
        cxT_sb = [mpool.tile([128, SKH], f16, tag=f"cxT{j}", name=f"cxT{j}")
                  for j in range(8)]

        # DRAM bounce buffers for the pairwise kv AllGather
        dpool = stageA.enter_context(
            tc.tile_pool(name="dramA", bufs=1, space="DRAM"))
        kv_in = dpool.tile([2 * SKH, D], f16, tag="kvin", name="kv_in")
        kv_out = dpool.tile([4 * SKH, D], f16, tag="kvout", name="kv_out")

        # sub-stage A1: raw x/cx tiles, freed after the transposes
        stageA1 = ExitStack()
        rpool = stageA1.enter_context(tc.tile_pool(name="rawA", bufs=1))
        x_sb = [rpool.tile([128, D], f16, tag=f"x{i}", name=f"x{i}")
                for i in range(4)]
        cx_sb = [rpool.tile([128, D], f16, tag=f"cx{i}", name=f"cx{i}")
                 for i in range(8)]
        for i in range(4):
            nc.sync.dma_start(x_sb[i][:, :], xin_d[128 * i:128 * (i + 1), :])
        for i in range(8):
            nc.sync.dma_start(
                cx_sb[i][:, :],
                xin_d[SQH + 128 * i:SQH + 128 * (i + 1), :])
        for j in range(8):
            for i in range(4):
                pt = psT.tile([128, 128], f16, tag="tp", name="tp")
                nc.tensor.transpose(
                    pt[:, :], x_sb[i][:, 128 * j:128 * (j + 1)], id_sb[:, :])
                nc.vector.tensor_copy(
                    xT_sb[j][:, 128 * i:128 * (i + 1)], pt[:, :])
        for j in range(8):
            for i in range(8):
                pt = psT.tile([128, 128], f16, tag="tp", name="tp")
                nc.tensor.transpose(
                    pt[:, :], cx_sb[i][:, 128 * j:128 * (j + 1)], id_sb[:, :])
                nc.vector.tensor_copy(
                    cxT_sb[j][:, 128 * i:128 * (i + 1)], pt[:, :])
        stageA1.close()

        # sub-stage A2: projection weights
        stageA2 = ExitStack()
        wpool = stageA2.enter_context(tc.tile_pool(name="wA", bufs=1))
        wq_sb = [wpool.tile([128, D], f16, tag=f"wq{d}", name=f"wq{d}")
                 for d in range(8)]
        wk_sb = [wpool.tile([128, D], f16, tag=f"wk{d}", name=f"wk{d}")
                 for d in range(8)]
        wv_sb = [wpool.tile([128, D], f16, tag=f"wv{d}", name=f"wv{d}")
                 for d in range(8)]
        for d in range(8):
            nc.sync.dma_start(wq_sb[d][:, :], wq_d[128 * d:128 * (d + 1), :])
            nc.sync.dma_start(wk_sb[d][:, :], wk_d[128 * d:128 * (d + 1), :])
            nc.sync.dma_start(wv_sb[d][:, :], wv_d[128 * d:128 * (d + 1), :])

        # qT tiles [128, 512]: tile t = features 128t..128t+127
        for t in range(8):
            ps = psP.tile([128, 512], f32, tag="proj", name="proj_ps")
            for d in range(8):
                nc.tensor.matmul(
                    ps[:, :],
                    lhsT=wq_sb[d][:, 128 * t:128 * (t + 1)],
                    rhs=xT_sb[d][:, :],
                    start=(d == 0), stop=(d == 7),
                )
            nc.vector.tensor_scalar_add(
                q_sb[t][:, :], ps[:, :], bq_sb[:, t:t + 1])
        # own-half kT tiles [128, 1024] -> DRAM bounce rows [128t, :]
        hpool = stageA2.enter_context(tc.tile_pool(name="halves", bufs=2))
        for t in range(8):
            kh = hpool.tile([128, SKH], f16, tag="kh", name="kh")
            for n in range(2):
                ps = psP.tile([128, 512], f32, tag="proj", name="proj_ps")
                for d in range(8):
                    nc.tensor.matmul(
                        ps[:, :],
                        lhsT=wk_sb[d][:, 128 * t:128 * (t + 1)],
                        rhs=cxT_sb[d][:, 512 * n:512 * (n + 1)],
                        start=(d == 0), stop=(d == 7),
                    )
                nc.vector.tensor_scalar_add(
                    kh[:, 512 * n:512 * (n + 1)], ps[:, :],
                    bk_sb[:, t:t + 1])
            nc.sync.dma_start(kv_in[128 * t:128 * (t + 1), :], kh[:, :])
        # own-half v tiles [128 skh, 1024] -> bounce rows [SKH + 128s, :]
        for s in range(8):
            vh = hpool.tile([128, D], f16, tag="vh", name="vh")
            for dc in range(2):
                ps = psP.tile([128, 512], f32, tag="proj", name="proj_ps")
                for d in range(8):
                    nc.tensor.matmul(
                        ps[:, :],
                        lhsT=cxT_sb[d][:, 128 * s:128 * (s + 1)],
                        rhs=wv_sb[d][:, 512 * dc:512 * (dc + 1)],
                        start=(d == 0), stop=(d == 7),
                    )
                nc.vector.tensor_add(
                    vh[:, 512 * dc:512 * (dc + 1)], ps[:, :],
                    bv_sb[:, 512 * dc:512 * (dc + 1)])
            nc.sync.dma_start(kv_in[SKH + 128 * s:SKH + 128 * (s + 1), :],
                              vh[:, :])

        # pairwise exchange: cores (2b, 2b+1) gather both kv halves
        nc.gpsimd.collective_compute(
            "AllGather",
            mybir.AluOpType.bypass,
            replica_groups=[[0, 1], [2, 3], [4, 5], [6, 7]],
            ins=[kv_in.opt()],
            outs=[kv_out.opt()],
        )

        # load back full kT [128, 2048] and v [128 sk, 1024] tiles
        for t in range(8):
            for r in range(2):
                nc.sync.dma_start(
                    k_sb[t][:, SKH * r:SKH * (r + 1)],
                    kv_out[2 * SKH * r + 128 * t:
                           2 * SKH * r + 128 * (t + 1), :])
        for s in range(NKC):
            r, sl = s // 8, s % 8
            nc.sync.dma_start(
                v_sb[s][:, :],
                kv_out[2 * SKH * r + SKH + 128 * sl:
                       2 * SKH * r + SKH + 128 * (sl + 1), :])

        stageA2.close()
        stageA.close()   # free xT/cxT/weights SBUF + psum pools

        # ---------------- Phase B: attention per head-pair ----------------
        stageB = ExitStack()
        work = stageB.enter_context(tc.tile_pool(name="work", bufs=3))
        norm = stageB.enter_context(tc.tile_pool(name="norm", bufs=1))
        psS = stageB.enter_context(
            tc.tile_pool(name="psS", bufs=3, space="PSUM"))
        psAcc = stageB.enter_context(
            tc.tile_pool(name="psAcc", bufs=2, space="PSUM"))
        psSum = stageB.enter_context(
            tc.tile_pool(name="psSum", bufs=2, space="PSUM"))

        for hp in range(8):
            acc = psAcc.tile([128, SQH], f32, tag="apsum", name="apsum")
            sums = work.tile([128, 2 * SQH], f16, tag="sums", name="sums",
                             bufs=2)
            for c in range(NKC):
                pt = work.tile([128, 2 * SQH], f16, tag="p", name="ptile")
                for a in range(2):
                    sc = psS.tile([128, SQH], f32, tag="scores",
                                  name="scores")
                    nc.tensor.matmul(
                        sc[:, :],
                        lhsT=k_sb[hp][64 * a:64 * (a + 1),
                                      128 * c:128 * (c + 1)],
                        rhs=q_sb[hp][64 * a:64 * (a + 1), :],
                        tile_position=(64 * a, 0),
                        start=True, stop=True,
                    )
                    nc.scalar.activation(
                        pt[:, SQH * a:SQH * (a + 1)], sc[:, :],
                        AF.Exp, bias=mk_sb[:, c:c + 1], scale=0.125,
                    )
                if c == 0:
                    nc.vector.tensor_copy(sums[:, :], pt[:, :])
                else:
                    nc.vector.tensor_add(sums[:, :], sums[:, :], pt[:, :])
                for a in range(2):
                    nc.tensor.matmul(
                        acc[64 * a:64 * (a + 1), :],
                        lhsT=v_sb[c][:, 64 * (2 * hp + a):
                                     64 * (2 * hp + a + 1)],
                        rhs=pt[:, SQH * a:SQH * (a + 1)],
                        tile_position=(0, 64 * a),
                        start=(c == 0), stop=(c == NKC - 1),
                    )
            # normalization: R = 1 / colsum(exp) broadcast to partitions
            r_bc = norm.tile([128, 2 * SQH], f32, tag="rbc", name="rbc")
            for j in range(2):
                sp = psSum.tile([128, 512], f32, tag="sum_ps", name="sum_ps")
                nc.tensor.matmul(
                    sp[:, :], lhsT=om_sb[:, :],
                    rhs=sums[:, 512 * j:512 * (j + 1)],
                    start=True, stop=True,
                )
                nc.vector.reciprocal_approx_fast(
                    r_bc[:, 512 * j:512 * (j + 1)], sp[:, :])
            for a in range(2):
                nc.vector.tensor_mul(
                    a_sb[hp][64 * a:64 * (a + 1), :],
                    acc[64 * a:64 * (a + 1), :],
                    r_bc[64 * a:64 * (a + 1), SQH * a:SQH * (a + 1)],
                )
        stageB.close()

        # ---------------- Phase C: c_proj, natural [sq, d] layout ---------
        stageC = ExitStack()
        cpool = stageC.enter_context(tc.tile_pool(name="stageC", bufs=2))
        psO = stageC.enter_context(
            tc.tile_pool(name="psO", bufs=2, space="PSUM"))
        for sqc in range(4):
            ot = cpool.tile([128, D], f16, tag="outT", name="ot")
            for dc in range(2):
                ps = psO.tile([128, 512], f32, tag="ops", name="ops")
                for hp in range(8):
                    nc.tensor.matmul(
                        ps[:, :],
                        lhsT=a_sb[hp][:, 128 * sqc:128 * (sqc + 1)],
                        rhs=wp_sb[2 * hp + dc][:, :],
                        start=(hp == 0), stop=(hp == 7),
                    )
                nc.vector.tensor_add(
                    ot[:, 512 * dc:512 * (dc + 1)], ps[:, :],
                    bp_sb[:, 512 * dc:512 * (dc + 1)])
            nc.sync.dma_start(out_d[128 * sqc:128 * (sqc + 1), :], ot[:, :])
        stageC.close()

    nc.compile()
    return nc


class _Runner:
    """Cached jit(shard_map(bass_exec)) with device-resident weights."""

    def __init__(self, nc):
        import jax
        import concourse.mybir as mybir
        from concourse import bass2jax
        from jax.sharding import Mesh, NamedSharding, PartitionSpec
        from jax.experimental.shard_map import shard_map

        bass2jax.install_neuronx_cc_hook()
        self.jax = jax
        self.nc = nc

        partition_name = (nc.partition_id_tensor.name
                          if nc.partition_id_tensor else None)
        in_names, out_names, out_avals = [], [], []
        for alloc in nc.m.functions[0].allocations:
            if not isinstance(alloc, mybir.MemoryLocationSet):
                continue
            name = alloc.memorylocations[0].name
            if alloc.kind == "ExternalInput":
                if name != partition_name:
                    in_names.append(name)
            elif alloc.kind == "ExternalOutput":
                out_names.append(name)
                out_avals.append(jax.core.ShapedArray(
                    tuple(alloc.tensor_shape), mybir.dt.np(alloc.dtype)))
        self.in_names = in_names
        self.out_names = out_names
        self.out_avals = out_avals
        n_params = len(in_names)
        bind_names = tuple(in_names + out_names +
                           ([partition_name] if partition_name else []))

        devices = jax.devices()[:NCORES]
        self.mesh = Mesh(np.asarray(devices), ("core",))
        self.shard = NamedSharding(self.mesh, PartitionSpec("core"))

        def _body(*args):
            operands = list(args)
            if partition_name is not None:
                operands.append(bass2jax.partition_id_tensor())
            outs = bass2jax._bass_exec_p.bind(
                *operands,
                out_avals=tuple(out_avals),
                in_names=bind_names,
                out_names=tuple(out_names),
                lowering_input_output_aliases=(),
                sim_require_finite=True,
                sim_require_nnan=True,
                nc=nc,
            )
            return tuple(outs)

        donate = tuple(range(n_params, n_params + len(out_names)))
        self.sharded = jax.jit(
            shard_map(_body, mesh=self.mesh,
                      in_specs=(PartitionSpec("core"),) * (n_params +
                                                           len(out_names)),
                      out_specs=(PartitionSpec("core"),) * len(out_names),
                      check_rep=False),
            donate_argnums=donate, keep_unused=True)

        import jax.numpy as jnp
        zero_shards = [NamedSharding(self.mesh, PartitionSpec("core"))
                       for _ in out_avals]

        def _mk_zeros():
            return tuple(
                jnp.zeros((NCORES * a.shape[0],) + tuple(a.shape[1:]),
                          a.dtype)
                for a in out_avals)

        self.mk_zeros = jax.jit(_mk_zeros, out_shardings=tuple(zero_shards))
        self._prev_outs = None

    def put(self, arr):
        """Host concat array -> device-resident sharded jax array."""
        return self.jax.device_put(arr, self.shard)

    def run(self, operands):
        """operands: dict name -> array (jax device or numpy concat)."""
        # Donation buffers: reuse the previous call's (already-fetched)
        # device outputs when available — the kernel overwrites every
        # element, so contents are irrelevant; else make zeros on-device.
        prev = self._prev_outs
        self._prev_outs = None
        if prev is None or any(p.is_deleted() for p in prev):
            prev = list(self.mk_zeros())
        args = [operands[n] for n in self.in_names] + prev
        outs = self.sharded(*args)
        self._prev_outs = list(outs)
        return outs


def _checksum(arr):
    """Full-integrity checksum (uint32-view sum) — cheap vs the transfer."""
    a = np.ascontiguousarray(arr)
    return (a.shape, a.dtype.str,
            int(a.view(np.uint32).sum(dtype=np.uint64)))


def _prep_weights(runner, Wq, bq, Wkv, bkv, Wp, bp):
    """Device-resident per-core weight shards (identical on all cores)."""
    f16, f32 = np.float16, np.float32

    def rep(a):   # replicate per core along axis0 for shard_map concat
        return np.ascontiguousarray(
            np.broadcast_to(a, (NCORES,) + a.shape)).reshape(
                (NCORES * a.shape[0],) + a.shape[1:])

    wq = Wq.astype(f16)
    wk = Wkv[:, :D].astype(f16)
    wv = Wkv[:, D:].astype(f16)
    wp = Wp.astype(f16)
    bq_t = np.ascontiguousarray(bq.astype(f32).reshape(8, 128).T)
    bk_t = np.ascontiguousarray(bkv[:D].astype(f32).reshape(8, 128).T)
    bv_bc = np.ascontiguousarray(
        np.broadcast_to(bkv[D:].astype(f32), (128, D)))
    bp_bc = np.ascontiguousarray(
        np.broadcast_to(bp.astype(f32), (128, D)))
    ident = np.eye(128, dtype=f16)
    ones_m = np.ones((128, 128), dtype=f16)
    host = {
        "wq": rep(wq), "wk": rep(wk), "wv": rep(wv), "wp": rep(wp),
        "bq_t": rep(bq_t), "bk_t": rep(bk_t), "bv_bc": rep(bv_bc),
        "bp_bc": rep(bp_bc), "ident": rep(ident), "ones_m": rep(ones_m),
    }
    return {k: runner.put(v) for k, v in host.items()}


def _pool():
    from concurrent.futures import ThreadPoolExecutor
    if "pool" not in _CACHE:
        _CACHE["pool"] = ThreadPoolExecutor(8)
    return _CACHE["pool"]


def _prep_acts(runner, x, ctx, attention_mask):
    """Per-call activations: pack each core's x+ctx contiguously so every
    device's complete input lands early in the upload stream (enables
    early per-core exec + output fetch overlap)."""
    f16, f32 = np.float16, np.float32
    x = np.asarray(x)
    ctx = np.asarray(ctx)
    SKH = SK // 2
    pack = np.empty((NCORES, SQH + SKH, D), f16)
    xv = x.reshape(NCORES, SQH, D)
    cv = ctx.reshape(NCORES, SKH, D)

    def fill(c):
        pack[c, :SQH] = xv[c]          # f32 -> f16 cast into place
        pack[c, SQH:] = cv[c]

    list(_pool().map(fill, range(NCORES)))
    m = np.asarray(attention_mask, dtype=f32).reshape(B, SK)
    mask_t = np.ascontiguousarray(
        m.reshape(B, NKC, 128).transpose(0, 2, 1))       # [4,128,16]
    concat_mask = np.ascontiguousarray(
        mask_t[[0, 0, 1, 1, 2, 2, 3, 3]]).reshape(NCORES * 128, NKC)
    return {"xin": pack.reshape(NCORES * (SQH + SKH), D),
            "mask_t": concat_mask}


def kernel(x, ctx, attention_mask, Wq, bq, Wkv, bkv, Wp, bp, **_ignored):
    x = np.asarray(x); ctx = np.asarray(ctx)
    attention_mask = np.asarray(attention_mask)
    Wq = np.asarray(Wq); bq = np.asarray(bq)
    Wkv = np.asarray(Wkv); bkv = np.asarray(bkv)
    Wp = np.asarray(Wp); bp = np.asarray(bp)

    if "nc" not in _CACHE:
        _CACHE["nc"] = _build_program()
    nc = _CACHE["nc"]
    if "runner" not in _CACHE:
        _CACHE["runner"] = _Runner(nc)
    runner = _CACHE["runner"]

    wkey = tuple(_checksum(a) for a in (Wq, bq, Wkv, bkv, Wp, bp))
    if _CACHE.get("wkey") != wkey:
        _CACHE["weights"] = _prep_weights(runner, Wq, bq, Wkv, bkv, Wp, bp)
        _CACHE["wkey"] = wkey

    operands = dict(_CACHE["weights"])
    operands.update(_prep_acts(runner, x, ctx, attention_mask))
    outs = runner.run(operands)
    try:
        # start the D2H stream the moment the device finishes, instead of
        # paying a client round trip after readiness
        outs[0].copy_to_host_async()
    except Exception:
        pass
    # fetch per-shard on threads; the f16->f32 cast of each shard overlaps
    # the remaining shard transfers
    out = np.empty((B, SQ, D), np.float32)
    flat = out.reshape(NCORES, SQH, D)
    try:
        shards = sorted(outs[0].addressable_shards,
                        key=lambda s: s.index[0].start or 0)
        if len(shards) != NCORES:
            raise ValueError("unexpected shard count")

        def fetch_one(i):
            flat[i] = np.asarray(shards[i].data)     # f16 -> f32 in place

        list(_pool().map(fetch_one, range(NCORES)))
    except Exception:
        out16 = np.asarray(outs[0])                      # fallback
        flat[:] = out16.reshape(NCORES, SQH, D)
    return out
